# revision 1
# baseline (speedup 1.0000x reference)
"""Trainium2 Bass kernel: forward kinematics of a 32-link serial chain.

Reference computes, per batch element b (262144 of them), a sequential scan
over 32 links maintaining a world rotation R and translation t, emitting
per-link [t, quat(R)] (quat in copysign/canonical form, w >= 0).

Device algorithm (all elementwise over the batch):
  - State is (quat q_cum, t) instead of a 3x3 matrix: cheaper compose.
  - Per link, q_link(theta) = E_b * sin(theta/2 + phi_b), b=0..3, with E/phi
    precomputed on host from the link's fixed rotation + joint axis
    (q_link = q_fixed x q_axis(theta) is affine in (sin(t/2), cos(t/2))).
  - q_cum' = q_cum x q_link: 16 scalar_tensor_tensor products (sign and E
    folded into the immediate scalar) + 12 adds.
  - t' = t + rot(q_cum, tf): via a = u x tf + w*tf (constant-coefficient
    combos -> STT chains), b = u x a, t' = t + tf + 2b.
  - Output canonicalization: multiply quat by Sign(w) (Abs for w itself),
    matching the reference's w>=0 convention.

Sharding: pure batch data-parallel across 8 cores (32768 batch each), one
[128 x 256] SBUF megatile per core; the interleaved [128, 256, 7] output
tile doubles as the recurrence state, DMA'd out per link.
"""
import sys
import numpy as np

for _p in ("/opt/trn_rl_repo", "/root/.axon_site/_ro/trn_rl_repo"):
    if _p not in sys.path:
        sys.path.append(_p)

P = 128
L = 32
B_TOTAL = 262144
N_CORES = 8
B_CORE = B_TOTAL // N_CORES      # 32768
F = B_CORE // P                  # 256

# quaternion product q_new_i = sum_k sign * qold_a * p_b   (x,y,z,w = 0..3)
_PROD = [
    (0, 3, 0, +1.0), (0, 0, 3, +1.0), (0, 1, 2, +1.0), (0, 2, 1, -1.0),
    (1, 3, 1, +1.0), (1, 0, 2, -1.0), (1, 1, 3, +1.0), (1, 2, 0, +1.0),
    (2, 3, 2, +1.0), (2, 0, 1, +1.0), (2, 1, 0, -1.0), (2, 2, 3, +1.0),
    (3, 3, 3, +1.0), (3, 0, 0, -1.0), (3, 1, 1, -1.0), (3, 2, 2, -1.0),
]


def _quat_mul(a, b):
    ax, ay, az, aw = a[..., 0], a[..., 1], a[..., 2], a[..., 3]
    bx, by, bz, bw = b[..., 0], b[..., 1], b[..., 2], b[..., 3]
    return np.stack([
        aw * bx + ax * bw + ay * bz - az * by,
        aw * by - ax * bz + ay * bw + az * bx,
        aw * bz + ax * by - ay * bx + az * bw,
        aw * bw - ax * bx - ay * by - az * bz,
    ], axis=-1)


def _mat_to_quat(R):
    """Shepperd largest-pivot matrix->quat (x,y,z,w), float64, per-matrix."""
    out = np.zeros(R.shape[:-2] + (4,), dtype=np.float64)
    for idx in np.ndindex(R.shape[:-2]):
        m = R[idx].astype(np.float64)
        tr = m[0, 0] + m[1, 1] + m[2, 2]
        cand = np.array([1 + tr,
                         1 + m[0, 0] - m[1, 1] - m[2, 2],
                         1 - m[0, 0] + m[1, 1] - m[2, 2],
                         1 - m[0, 0] - m[1, 1] + m[2, 2]])
        p = int(np.argmax(cand))
        s = 0.5 * np.sqrt(cand[p])
        if p == 0:
            w, x = s, (m[2, 1] - m[1, 2]) / (4 * s)
            y, z = (m[0, 2] - m[2, 0]) / (4 * s), (m[1, 0] - m[0, 1]) / (4 * s)
        elif p == 1:
            x, w = s, (m[2, 1] - m[1, 2]) / (4 * s)
            y, z = (m[0, 1] + m[1, 0]) / (4 * s), (m[0, 2] + m[2, 0]) / (4 * s)
        elif p == 2:
            y, w = s, (m[0, 2] - m[2, 0]) / (4 * s)
            x, z = (m[0, 1] + m[1, 0]) / (4 * s), (m[1, 2] + m[2, 1]) / (4 * s)
        else:
            z, w = s, (m[1, 0] - m[0, 1]) / (4 * s)
            x, y = (m[0, 2] + m[2, 0]) / (4 * s), (m[1, 2] + m[2, 1]) / (4 * s)
        if w < 0:
            x, y, z, w = -x, -y, -z, -w
        out[idx] = (x, y, z, w)
    return out


def _build_constants(link_trans, link_rot, joint_axes):
    """Per link, q_link(theta)_b = A_b sin(theta/2) + B_b cos(theta/2).
    The ACT Sin table is only accurate for |arg| < pi, so the device computes
    the basis pair (s2 = sin(u), c2 = sin(pi/2 - |u|) = cos(u)) and forms
    p~_b = base_b + ratio_b * other_b with |ratio| <= 1; the larger coefficient
    C_b is folded into downstream STT immediates (q_link_b = C_b * p~_b).

    Returns C[L,4], ratio[L,4], use_cos_base[L,4] (bool), v[L,3]."""
    qf = _mat_to_quat(np.asarray(link_rot, dtype=np.float64))
    ax = np.asarray(joint_axes, dtype=np.float64)
    axq = np.concatenate([ax, np.zeros((L, 1))], axis=-1)
    A = _quat_mul(qf, axq)        # coefficient of sin(theta/2)
    Bc = qf                       # coefficient of cos(theta/2)
    use_cos = np.abs(Bc) >= np.abs(A)
    C = np.where(use_cos, Bc, A)
    safe = np.where(C == 0.0, 1.0, C)
    ratio = np.where(use_cos, A, Bc) / safe
    ratio = np.where(C == 0.0, 0.0, ratio)
    return C, ratio, use_cos, np.asarray(link_trans, dtype=np.float64)


def _a_chain_consts(v):
    """Per t-component i: sources (u_j, u_k, w) with coefs (v_k, -v_j, v_i).
    Returns for each i: (order of STT ops, base source, cm) with the largest
    |coef| term factored out (its source is the unscaled in1 seed)."""
    plans = []
    for i in range(3):
        j, k = (i + 1) % 3, (i + 2) % 3
        srcs = [3 + j, 3 + k, 6]        # out-tile comp indices (u_j, u_k, w)
        coefs = [v[k], -v[j], v[i]]
        im = int(np.argmax(np.abs(coefs)))
        cm = coefs[im]
        rest = [(srcs[n], coefs[n] / cm) for n in range(3) if n != im]
        plans.append((srcs[im], rest, cm))
    return plans


DEFAULT_CFG = {'x1': 'G', 'x2': 'V', 'qraw': 'V', 'bsub': 'G', 'st': 'G',
               'canon0': 'V', 'canon1': 'G', 'canon2': 'G',
               'bp0': 'G', 'bp1': 'G', 'bp2': 'G', 'bp3': 'G', 'bp4': 'G', 'bp5': 'G'}


def _emit(tc, q_ap, out_aps, C, ratio, use_cos, v64, mybir, cfg=None, reps=1):
    """Emit the per-core Tile program. q_ap: [B_CORE, 32] DRAM; out_aps[l]:
    [B_CORE, 7] DRAM per link."""
    nc = tc.nc
    cfg = cfg or dict(DEFAULT_CFG)
    E = lambda key: {'V': nc.vector, 'G': nc.gpsimd}[cfg[key]]
    f32 = mybir.dt.float32
    Alu = mybir.AluOpType
    Act = mybir.ActivationFunctionType
    from contextlib import ExitStack

    ctx = ExitStack()
    qpool = ctx.enter_context(tc.tile_pool(name="qin", bufs=1))
    outpool = ctx.enter_context(tc.tile_pool(name="out", bufs=3))
    ppool = ctx.enter_context(tc.tile_pool(name="p", bufs=2))
    spool = ctx.enter_context(tc.tile_pool(name="scratch", bufs=2))

    q_sb = qpool.tile([P, F * L], f32, tag="q_sb", name="q_sb")
    nc.sync.dma_start(q_sb[:], q_ap.rearrange("(p t) l -> p (t l)", p=P))
    q3 = q_sb[:].rearrange("p (t l) -> p t l", l=L)

    import contextlib
    loop_ctx = tc.For_i(0, reps, 1) if reps > 1 else contextlib.nullcontext()
    with loop_ctx:
      prev = None   # previous out-tile 3d view
      for l in range(L):
          a_plans = _a_chain_consts(v64[l])
          po = outpool.tile([P, F * 7], f32, tag="po", name="po")
          po3 = po[:].rearrange("p (t c) -> p t c", c=7)

          # --- basis pair s2 = sin(u), c2 = cos(u) = sin(pi/2 - |u|) ------
          au = spool.tile([P, F], f32, tag="au", name="au")
          s2 = ppool.tile([P, F], f32, tag="s2", name="s2")
          c2 = ppool.tile([P, F], f32, tag="c2", name="c2")
          nc.scalar.activation(au[:], q3[:, :, l], Act.Abs)
          nc.scalar.activation(s2[:], q3[:, :, l], Act.Sin)
          nc.scalar.activation(c2[:], au[:], Act.Sin,
                               bias=float(np.pi / 2), scale=-1.0)
          # --- p~_b = base + ratio * other  (q_link_b = C_b * p~_b) -------
          pt = ppool.tile([P, F * 4], f32, tag="pt", name="pt")
          pt3 = pt[:].rearrange("p (t c) -> p t c", c=4)
          for b in range(4):
              base = c2 if use_cos[l][b] else s2
              other = s2 if use_cos[l][b] else c2
              nc.vector.scalar_tensor_tensor(pt3[:, :, b], other[:],
                                             float(ratio[l][b]), base[:],
                                             Alu.mult, Alu.add)

          qraw = spool.tile([P, F * 4], f32, tag="qraw", name="qraw")
          qraw3 = qraw[:].rearrange("p (t c) -> p t c", c=4)

          if l == 0:
              for b in range(4):
                  nc.vector.tensor_scalar_mul(qraw3[:, :, b], pt3[:, :, b],
                                              float(C[0][b]))
              for i in range(3):
                  nc.gpsimd.memset(po3[:, :, i], float(v64[0][i]))
          else:
              # --- compose: 16 STT products into 4 term-group tiles, then
              #     pairwise big adds (scalar_tensor_tensor is DVE-only) ---
              gts = [spool.tile([P, F * 4], f32, tag=f"g{k}", name=f"g{k}") for k in range(4)]
              g3 = [g[:].rearrange("p (t c) -> p t c", c=4) for g in gts]
              for k, (i, a, b, s) in enumerate(_PROD):
                  # term j of output i goes to group tile j at slot i
                  nc.vector.scalar_tensor_tensor(g3[k % 4][:, :, i], pt3[:, :, b],
                                                 float(s * C[l][b]), prev[:, :, 3 + a],
                                                 Alu.mult, Alu.mult)
              x1 = spool.tile([P, F * 4], f32, tag="x1", name="x1")
              x2 = spool.tile([P, F * 4], f32, tag="x2", name="x2")
              E('x1').tensor_add(x1[:], gts[0][:], gts[1][:])
              E('x2').tensor_add(x2[:], gts[2][:], gts[3][:])
              E('qraw').tensor_add(qraw[:], x1[:], x2[:])
              # --- t update ----------------------------------------------
              at = spool.tile([P, F * 3], f32, tag="at", name="at")
              at3 = at[:].rearrange("p (t c) -> p t c", c=3)
              cms = []
              for i in range(3):
                  base, rest, cm = a_plans[i]
                  (s1, c1), (s2, c2) = rest
                  nc.vector.scalar_tensor_tensor(at3[:, :, i], prev[:, :, s1],
                                                 float(c1), prev[:, :, base],
                                                 Alu.mult, Alu.add)
                  nc.vector.scalar_tensor_tensor(at3[:, :, i], prev[:, :, s2],
                                                 float(c2), at3[:, :, i],
                                                 Alu.mult, Alu.add)
                  cms.append(cm)
              asc = spool.tile([P, F * 3], f32, tag="asc", name="asc")
              asc3 = asc[:].rearrange("p (t c) -> p t c", c=3)
              for i in range(3):
                  # asc_i = 2 * cm_i * at_i  (true a-component, x2 folded)
                  nc.scalar.activation(asc3[:, :, i], at3[:, :, i], Act.Copy,
                                       bias=0.0, scale=float(2.0 * cms[i]))
              bm1 = spool.tile([P, F * 3], f32, tag="bm1", name="bm1")
              bm13 = bm1[:].rearrange("p (t c) -> p t c", c=3)
              bm2 = spool.tile([P, F * 3], f32, tag="bm2", name="bm2")
              bm23 = bm2[:].rearrange("p (t c) -> p t c", c=3)
              for i in range(3):
                  j, k = (i + 1) % 3, (i + 2) % 3
                  # b2_i = u_j * (2 a_k) - u_k * (2 a_j)
                  E(f'bp{2*i}').tensor_mul(bm13[:, :, i], asc3[:, :, k],
                                           prev[:, :, 3 + j])
                  E(f'bp{2*i+1}').tensor_mul(bm23[:, :, i], asc3[:, :, j],
                                             prev[:, :, 3 + k])
              b2 = spool.tile([P, F * 3], f32, tag="b2", name="b2")
              E('bsub').tensor_sub(b2[:], bm1[:], bm2[:])
              st = spool.tile([P, F * 3], f32, tag="st", name="st")
              st3 = st[:].rearrange("p (t c) -> p t c", c=3)
              E('st').tensor_add(st3[:, :, :], b2[:].rearrange("p (t c) -> p t c", c=3),
                                  prev[:, :, 0:3])
              for i in range(3):
                  nc.scalar.activation(po3[:, :, i], st3[:, :, i], Act.Copy,
                                       bias=float(v64[l][i]), scale=1.0)

          # --- canonicalize + write quat ---------------------------------
          sg = spool.tile([P, F], f32, tag="sg", name="sg")
          nc.scalar.activation(sg[:], qraw3[:, :, 3], Act.Sign)
          for i in range(3):
              E(f'canon{i}').tensor_mul(po3[:, :, 3 + i], qraw3[:, :, i], sg[:])
          nc.scalar.activation(po3[:, :, 6], qraw3[:, :, 3], Act.Abs)

          nc.sync.dma_start(out_aps[l].rearrange("(p t) c -> p (t c)", p=P), po[:])
          prev = po3
    ctx.close()


def _build_program(C, ratio, use_cos, v64, cfg=None, reps=1):
    import concourse.tile as tile
    from concourse import bacc, mybir

    nc = bacc.Bacc("TRN2", target_bir_lowering=False, debug=False,
                   enable_asserts=False, num_devices=N_CORES)
    f32 = mybir.dt.float32

    # non-Copy activation float biases require pre-registered const APs
    for val in (float(np.pi / 2),):
        if (f32, val) not in nc.const_aps.aps:
            t = nc.alloc_sbuf_tensor(f"const-f32-{val}", [128, 1], f32)
            nc.gpsimd.memset(t.ap(), val)
            nc.const_aps.aps[(f32, val)] = t.ap()
    nc.all_engine_barrier()
    q_ap = nc.dram_tensor("q", [B_CORE, L], f32, kind="ExternalInput").ap()
    out_aps = [nc.dram_tensor(f"out{l}", [B_CORE, 7], f32, kind="ExternalOutput").ap()
               for l in range(L)]
    with tile.TileContext(nc) as tc:
        _emit(tc, q_ap, out_aps, C, ratio, use_cos, v64, mybir, cfg=cfg, reps=reps)
    nc.compile()
    return nc


TRACE = False      # set True (e.g. from test.py) to NTFF-profile the run
LAST = None        # BassKernelResults of the most recent kernel() call


def kernel(q, link_trans, link_rot, joint_axes):
    from concourse.bass_utils import run_bass_kernel_spmd

    C, ratio, use_cos, v64 = _build_constants(link_trans, link_rot, joint_axes)
    nc = _build_program(C, ratio, use_cos, v64)

    # host-side half-angle wrap keeps every ACT Sin argument within +-3pi/2
    qh = np.asarray(q, dtype=np.float32) * np.float32(0.5)
    qh = (qh + np.float32(np.pi)) % np.float32(2 * np.pi) - np.float32(np.pi)
    in_maps = [{"q": np.ascontiguousarray(qh[c * B_CORE:(c + 1) * B_CORE])}
               for c in range(N_CORES)]
    import time
    t0 = time.time()
    res = run_bass_kernel_spmd(nc, in_maps, list(range(N_CORES)))
    exec1 = time.time() - t0
    global LAST, EXEC_WALL_S
    LAST = res
    EXEC_WALL_S = exec1
    if TRACE:
        # warm second execution for a dispatch+exec wall-clock measurement
        t0 = time.time()
        res = run_bass_kernel_spmd(nc, in_maps, list(range(N_CORES)))
        EXEC_WALL_S = time.time() - t0
        LAST = res
    per_core = [np.stack([r[f"out{l}"] for l in range(L)], axis=0)
                for r in res.results]
    return np.concatenate(per_core, axis=1)



# revision 3
# speedup vs baseline: 2.5241x; 2.5241x over previous
"""Trainium2 Bass kernel: forward kinematics of a 32-link serial chain.

Layout: per core, partitions = quat comp c (0..3)*32 + batch group g (0..31);
free e = 0..1023; batch elem = g*1024 + e. Components live on PARTITIONS, so
every constant linear map runs on the (otherwise idle) TensorEngine as a
128x128 block-diagonal matmul in float32r (1 cycle/row, fp32 accumulate).

Per link l (Q = cumulative quat after link l-1; u = Q.xyz, w = Q.w):
  thB  = Wbc_j . th_quad           (PE: broadcast theta_l/2 to all 4 comps)
  au   = Abs(thB), sB = Sin(thB), cB = Sin(pi/2 - au)      (ACT, PSUM src)
  sQ   = sB*Qp,  cQ = cB*Qp        (DVE, reading the previous compose result
                                    straight from PSUM - keeps the loop-
                                    carried path off the state copy)
  Qp'  = W_A . sQ  (+)  W_B . cQ   (PE, PSUM-accumulated; legal because the
                                    per-element gates commute with the
                                    constant maps: s(QxA) = (sQ)xA)
  qt   = copy(Qp')                 (DVE; feeds t-part matmul rhs + DMA out)
  a    = W_a . Q   (a = u x v_l + w v_l), staged to SBUF    (PE + ACT copy)
  uP2  = W_u2 . Q  (uP2_i = u_{i+2}),  aP2 = W_a2 . Q  (aP2_i = a_{i+2})
  mx   = a * uP2,  my = Q * aP2    (DVE; rolled cross product:
                                    b~_i = b_{(i+1)%3} = u_{i+2}a_i - u_i a_{i+2})
  S   += mx; S -= my               (Pool; S = sum of rolled b's; t = C + 2S)
  tt   = 2*S + C_l                 (ACT Identity, per-partition rolled bias)
  out: qt (un-canonicalized; host flips where w<0 - legal since the rotation
       action is quadratic in Q), tt (host un-rolls the component slots)

DMA inside the link loop: only the two output stores. theta quads, PE weights
and bias columns are preloaded to SBUF; outputs are [L,128,1024] planes that
the host reassembles into [L, B, 7] (cheap numpy transpose + sign fix).
Sharding: pure batch data-parallel across 8 cores (32768 batch each).
"""
import sys
import numpy as np

for _p in ("/opt/trn_rl_repo", "/root/.axon_site/_ro/trn_rl_repo"):
    if _p not in sys.path:
        sys.path.append(_p)

P = 128
L = 32
B_TOTAL = 262144
N_CORES = 8
B_CORE = B_TOTAL // N_CORES      # 32768
G = 32                            # batch groups per core
E = B_CORE // G                   # 1024 free elems per partition


def _quat_mul(a, b):
    ax, ay, az, aw = a[..., 0], a[..., 1], a[..., 2], a[..., 3]
    bx, by, bz, bw = b[..., 0], b[..., 1], b[..., 2], b[..., 3]
    return np.stack([
        aw * bx + ax * bw + ay * bz - az * by,
        aw * by - ax * bz + ay * bw + az * bx,
        aw * bz + ax * by - ay * bx + az * bw,
        aw * bw - ax * bx - ay * by - az * bz,
    ], axis=-1)


def _mat_to_quat(R):
    """Shepperd largest-pivot matrix->quat (x,y,z,w), float64, per-matrix."""
    out = np.zeros(R.shape[:-2] + (4,), dtype=np.float64)
    for idx in np.ndindex(R.shape[:-2]):
        m = R[idx].astype(np.float64)
        tr = m[0, 0] + m[1, 1] + m[2, 2]
        cand = np.array([1 + tr,
                         1 + m[0, 0] - m[1, 1] - m[2, 2],
                         1 - m[0, 0] + m[1, 1] - m[2, 2],
                         1 - m[0, 0] - m[1, 1] + m[2, 2]])
        p = int(np.argmax(cand))
        s = 0.5 * np.sqrt(cand[p])
        if p == 0:
            w, x = s, (m[2, 1] - m[1, 2]) / (4 * s)
            y, z = (m[0, 2] - m[2, 0]) / (4 * s), (m[1, 0] - m[0, 1]) / (4 * s)
        elif p == 1:
            x, w = s, (m[2, 1] - m[1, 2]) / (4 * s)
            y, z = (m[0, 1] + m[1, 0]) / (4 * s), (m[0, 2] + m[2, 0]) / (4 * s)
        elif p == 2:
            y, w = s, (m[0, 2] - m[2, 0]) / (4 * s)
            x, z = (m[0, 1] + m[1, 0]) / (4 * s), (m[1, 2] + m[2, 1]) / (4 * s)
        else:
            z, w = s, (m[1, 0] - m[0, 1]) / (4 * s)
            x, y = (m[0, 2] + m[2, 0]) / (4 * s), (m[1, 2] + m[2, 1]) / (4 * s)
        if w < 0:
            x, y, z, w = -x, -y, -z, -w
        out[idx] = (x, y, z, w)
    return out


def _right_mult_matrix(Pq):
    """M with (Q x P) = M @ Q for constant P, Q column (x,y,z,w)."""
    Px, Py, Pz, Pw = Pq
    return np.array([
        [Pw,  Pz, -Py, Px],
        [-Pz, Pw,  Px, Py],
        [Py, -Px,  Pw, Pz],
        [-Px, -Py, -Pz, Pw],
    ])


def _a_matrix(v):
    """a = u x v + w v over Q=(x,y,z,w); row 3 zero."""
    v0, v1, v2 = v
    return np.array([
        [0.0,  v2, -v1, v0],
        [-v2, 0.0,  v0, v1],
        [v1, -v0, 0.0,  v2],
        [0.0, 0.0, 0.0, 0.0],
    ])


def _blockdiag(M):
    """[128,128] lhsT for blockwise out[i*32+g] = sum_a M[i,a] in[a*32+g]:
    lhsT[k=a*32+g, m=i*32+g] = M[i,a]."""
    blk = np.zeros((128, 128), dtype=np.float64)
    for a in range(4):
        for i in range(4):
            if M[i, a] != 0.0:
                idx = np.arange(G)
                blk[a * G + idx, i * G + idx] = M[i, a]
    return blk


def _build_constants(link_trans, link_rot, joint_axes):
    qf = _mat_to_quat(np.asarray(link_rot, dtype=np.float64))
    ax = np.asarray(joint_axes, dtype=np.float64)
    axq = np.concatenate([ax, np.zeros((L, 1))], axis=-1)
    A = _quat_mul(qf, axq)          # coef of sin(theta/2)
    Bq = qf                         # coef of cos(theta/2)
    v = np.asarray(link_trans, dtype=np.float64)

    # per-link weights: [L, 4, 128, 128]: W_A, W_B, W_a, W_a2
    roll2 = np.zeros((4, 4))
    for i in range(3):
        roll2[i, (i + 2) % 3] = 1.0
    wm = np.zeros((L, 4, 128, 128), dtype=np.float32)
    for l in range(L):
        Ma = _a_matrix(v[l])
        mats = [_right_mult_matrix(A[l]), _right_mult_matrix(Bq[l]),
                Ma, roll2 @ Ma]
        for widx, M in enumerate(mats):
            wm[l, widx] = _blockdiag(M).astype(np.float32)

    # link-independent weights: Wbc_j (j=0..3) and W_u2: [5, 128, 128]
    wfix = np.zeros((5, 128, 128), dtype=np.float32)
    for j in range(4):
        Mb = np.zeros((4, 4))
        Mb[:, j] = 1.0              # out[i] = in[j] for all i
        wfix[j] = _blockdiag(Mb).astype(np.float32)
    Mu2 = np.zeros((4, 4))
    for i in range(3):
        Mu2[i, (i + 2) % 3] = 1.0   # uP2_i = u_{i+2}, row3 = 0
    wfix[4] = _blockdiag(Mu2).astype(np.float32)

    # consts [128, 33]: col l = rolled C_l bias (slot i = C_l[(i+1)%3]);
    # col 32 = identity quat
    consts = np.zeros((128, L + 1), dtype=np.float32)
    Cl = np.zeros(3)
    for l in range(L):
        Cl += v[l]
        for i in range(3):
            consts[i * G:(i + 1) * G, l] = Cl[(i + 1) % 3]
    consts[3 * G:4 * G, L] = 1.0
    return A, Bq, v, wm, wfix, consts


DEFAULT_CFG = {'saccx': 'G', 'saccy': 'G', 'sq': 'V', 'cq': 'V',
               'qcopy': 'V', 'acopy': 'A'}


def _emit(tc, aps, mybir, cfg=None, reps=1):
    nc = tc.nc
    cfg = cfg or dict(DEFAULT_CFG)
    Eng = lambda key: {'V': nc.vector, 'G': nc.gpsimd}[cfg[key]]
    f32 = mybir.dt.float32
    f32r = mybir.dt.float32r
    Act = mybir.ActivationFunctionType
    qT_ap, wm_ap, wfix_ap, cst_ap, outq_ap, outt_ap = aps
    from contextlib import ExitStack

    H = E // 2   # 512

    ctx = ExitStack()
    thp = ctx.enter_context(tc.tile_pool(name="th", bufs=1))
    wp = ctx.enter_context(tc.tile_pool(name="wts", bufs=1))
    cstp = ctx.enter_context(tc.tile_pool(name="cst", bufs=1))
    bas = ctx.enter_context(tc.tile_pool(name="basis", bufs=2))
    gp = ctx.enter_context(tc.tile_pool(name="gated", bufs=2))
    mpx = ctx.enter_context(tc.tile_pool(name="mx", bufs=2))
    qp = ctx.enter_context(tc.tile_pool(name="q", bufs=3))
    tp = ctx.enter_context(tc.tile_pool(name="t", bufs=3))
    sp = ctx.enter_context(tc.tile_pool(name="s", bufs=1))
    ps1 = ctx.enter_context(tc.tile_pool(name="ps1", bufs=1, space="PSUM"))
    psq = ctx.enter_context(tc.tile_pool(name="psq", bufs=2, space="PSUM"))
    ps2 = ctx.enter_context(tc.tile_pool(name="ps2", bufs=1, space="PSUM"))

    cst = cstp.tile([128, L + 1], f32, tag="cst", name="cst")
    nc.sync.dma_start(cst[:], cst_ap)
    th = thp.tile([128, 8 * E], f32r, tag="th", name="th")
    for lq in range(8):
        nc.sync.dma_start(th[:, lq * E:(lq + 1) * E],
                          qT_ap[4 * lq:4 * lq + 4, :].rearrange(
                              "j (g e) -> (j g) e", e=E))
    # weights: per-link [L, 4] at wsb[:, (l*4+widx)*128 : ...], then 5 fixed
    wsb = wp.tile([128, (4 * L + 5) * 128], f32r, tag="wsb", name="wsb")
    for l in range(L):
        nc.sync.dma_start(
            wsb[:, l * 512:(l + 1) * 512].rearrange("k (w m) -> k w m", m=128),
            wm_ap[l].rearrange("w k m -> k w m"))
    nc.sync.dma_start(
        wsb[:, 4 * L * 128:].rearrange("k (w m) -> k w m", m=128),
        wfix_ap.rearrange("w k m -> k w m"))

    def W(l, widx):
        off = (l * 4 + widx) * 128
        return wsb[:, off:off + 128]

    def Wfix(j):
        off = (4 * L + j) * 128
        return wsb[:, off:off + 128]

    import contextlib
    loop_ctx = tc.For_i(0, reps, 1) if reps > 1 else contextlib.nullcontext()
    with loop_ctx:
        qinit = sp.tile([128, E], f32r, tag="qinit", name="qinit")
        nc.scalar.activation(qinit[:], th[:, 0:E], Act.Identity,
                             bias=cst[:, L:L + 1], scale=0.0)
        S = sp.tile([128, E], f32, tag="S", name="S")
        nc.gpsimd.memset(S[:], 0.0)

        prev = qinit
        prev_ps = None
        for l in range(L):
            lq, jj = divmod(l, 4)
            ths = th[:, lq * E:(lq + 1) * E]

            # ---- basis: thB = bcast_j(theta quad) on PE; Abs/Sin on ACT ----
            thB = ps1.tile([128, E], f32, tag="thB", name="thB")
            for h in range(2):
                sl = slice(h * H, (h + 1) * H)
                nc.tensor.matmul(thB[:, sl], Wfix(jj), ths[:, sl],
                                 start=True, stop=True)
            au = bas.tile([128, E], f32, tag="au", name="au")
            sB = bas.tile([128, E], f32, tag="sB", name="sB")
            cB = bas.tile([128, E], f32, tag="cB", name="cB")
            nc.scalar.activation(au[:], thB[:], Act.Abs)
            nc.scalar.activation(sB[:], thB[:], Act.Sin)
            nc.scalar.activation(cB[:], au[:], Act.Sin,
                                 bias=float(np.pi / 2), scale=-1.0)

            # ---- compose: Qp = W_A.(sB*Q) + W_B.(cB*Q), PSUM-accumulated ----
            sQ = gp.tile([128, E], f32r, tag="sQ", name="sQ")
            cQ = gp.tile([128, E], f32r, tag="cQ", name="cQ")
            comp_src = prev_ps[:] if prev_ps is not None else prev[:].bitcast(f32)
            Eng('sq').tensor_mul(sQ[:], sB[:], comp_src)
            Eng('cq').tensor_mul(cQ[:], cB[:], comp_src)
            Qp = psq.tile([128, E], f32, tag="Qp", name="Qp")
            for h in range(2):
                sl = slice(h * H, (h + 1) * H)
                nc.tensor.matmul(Qp[:, sl], W(l, 0), sQ[:, sl],
                                 start=True, stop=False)
                nc.tensor.matmul(Qp[:, sl], W(l, 1), cQ[:, sl],
                                 start=False, stop=True)
            qt = qp.tile([128, E], f32r, tag="qt", name="qt")
            if cfg['qcopy'] == 'A':
                nc.scalar.copy(qt[:], Qp[:])
            else:
                Eng('qcopy').tensor_copy(qt[:], Qp[:])

            # ---- t part: rolled cross product off Q_{l-1} ----
            a_sb = gp.tile([128, E], f32, tag="a_sb", name="a_sb")
            for h in range(2):
                sl = slice(h * H, (h + 1) * H)
                a_ps = ps1.tile([128, H], f32, tag="a", name="a")
                nc.tensor.matmul(a_ps[:], W(l, 2), prev[:, sl],
                                 start=True, stop=True)
                if cfg['acopy'] == 'A':
                    nc.scalar.copy(a_sb[:, sl], a_ps[:])
                else:
                    nc.vector.tensor_copy(a_sb[:, sl], a_ps[:])
            mx = mpx.tile([128, E], f32, tag="mxt", name="mxt")
            my = mpx.tile([128, E], f32, tag="myt", name="myt")
            for h in range(2):
                sl = slice(h * H, (h + 1) * H)
                u2h = ps2.tile([128, H], f32, tag="t2", name="u2")
                nc.tensor.matmul(u2h[:], Wfix(4), prev[:, sl],
                                 start=True, stop=True)
                nc.vector.tensor_mul(mx[:, sl], a_sb[:, sl], u2h[:])
                a2h = ps2.tile([128, H], f32, tag="t2", name="a2")
                nc.tensor.matmul(a2h[:], W(l, 3), prev[:, sl],
                                 start=True, stop=True)
                nc.vector.tensor_mul(my[:, sl], prev[:, sl].bitcast(f32), a2h[:])
            Eng('saccx').tensor_add(S[:], S[:], mx[:])
            Eng('saccy').tensor_sub(S[:], S[:], my[:])

            tt = tp.tile([128, E], f32, tag="tt", name="tt")
            nc.scalar.activation(tt[:], S[:], Act.Identity,
                                 bias=cst[:, l:l + 1], scale=2.0)

            nc.sync.dma_start(outq_ap[l], qt[:])
            nc.sync.dma_start(outt_ap[l], tt[:])
            prev = qt
            prev_ps = Qp
    ctx.close()


def _build_program(consts_tuple, cfg=None, reps=1):
    import concourse.tile as tile
    from concourse import bacc, mybir

    A, Bq, v, wm, wfix, consts = consts_tuple
    nc = bacc.Bacc("TRN2", target_bir_lowering=False, debug=False,
                   enable_asserts=False, num_devices=N_CORES)
    f32 = mybir.dt.float32

    for val in (float(np.pi / 2),):
        if (f32, val) not in nc.const_aps.aps:
            t = nc.alloc_sbuf_tensor(f"const-f32-{val}", [128, 1], f32)
            nc.gpsimd.memset(t.ap(), val)
            nc.const_aps.aps[(f32, val)] = t.ap()
    nc.all_engine_barrier()

    f32r = mybir.dt.float32r
    qT_ap = nc.dram_tensor("qT", [L, B_CORE], f32r, kind="ExternalInput").ap()
    wm_ap = nc.dram_tensor("wm", [L, 4, 128, 128], f32r, kind="ExternalInput").ap()
    wfix_ap = nc.dram_tensor("wfix", [5, 128, 128], f32r, kind="ExternalInput").ap()
    cst_ap = nc.dram_tensor("consts", [128, L + 1], f32, kind="ExternalInput").ap()
    outq_ap = nc.dram_tensor("outq", [L, 128, E], f32r, kind="ExternalOutput").ap()
    outt_ap = nc.dram_tensor("outt", [L, 128, E], f32, kind="ExternalOutput").ap()
    with tile.TileContext(nc) as tc:
        _emit(tc, (qT_ap, wm_ap, wfix_ap, cst_ap, outq_ap, outt_ap), mybir,
              cfg=cfg, reps=reps)
    nc.compile()
    return nc


def prepare_in_maps(q, consts_tuple):
    A, Bq, v, wm, wfix, consts = consts_tuple
    qh = np.asarray(q, dtype=np.float32) * np.float32(0.5)
    qh = (qh + np.float32(np.pi)) % np.float32(2 * np.pi) - np.float32(np.pi)
    in_maps = []
    for c in range(N_CORES):
        qT = np.ascontiguousarray(qh[c * B_CORE:(c + 1) * B_CORE].T)
        in_maps.append({"qT": qT, "wm": wm, "wfix": wfix, "consts": consts})
    return in_maps


def assemble_output(results):
    out = np.empty((L, B_TOTAL, 7), dtype=np.float32)
    for c, r in enumerate(results):
        sl = slice(c * B_CORE, (c + 1) * B_CORE)
        qa = r["outq"].reshape(L, 4, G, E)
        ta = r["outt"].reshape(L, 4, G, E)
        # slot i holds t_{(i+1)%3}: t_j lives at slot (j-1)%3 = (j+2)%3
        tfix = ta[:, [2, 0, 1]]     # tfix[:, j] = t_j
        out[:, sl, 0:3] = tfix.transpose(0, 2, 3, 1).reshape(L, B_CORE, 3)
        out[:, sl, 3:7] = qa.transpose(0, 2, 3, 1).reshape(L, B_CORE, 4)
    neg = out[:, :, 6] < 0
    out[:, :, 3:7][neg] *= -1.0
    return out


TRACE = False
LAST = None


def kernel(q, link_trans, link_rot, joint_axes):
    from concourse.bass_utils import run_bass_kernel_spmd

    ct = _build_constants(link_trans, link_rot, joint_axes)
    nc = _build_program(ct)
    in_maps = prepare_in_maps(q, ct)
    import time
    t0 = time.time()
    res = run_bass_kernel_spmd(nc, in_maps, list(range(N_CORES)))
    global LAST, EXEC_WALL_S
    LAST = res
    EXEC_WALL_S = time.time() - t0
    return assemble_output(res.results)
